# revision 50
# baseline (speedup 1.0000x reference)
"""Causal multi-head attention (B=2, S=2048, D=1024, H=16, hd=64) on 8 trn2 cores.

Sharding: core = (batch b, head-group g): cores 0-3 -> batch 0, groups 0-3;
cores 4-7 -> batch 1. Each core computes 4 heads of one batch element:
QKV projections for its 256 hd-dims, causal attention, and a partial output
projection (attn_heads @ Wo.T restricted to its hd columns). Host sums the 4
partials per batch (bf16 partials) and adds bo.

v2 changes vs the 208us baseline:
- Scores row-tiled: each head's K^T chunk is only 64 contraction rows, so the
  two heads of a pair run CONCURRENTLY on the PE via tile_position (0,0) and
  (64,0) into separate PSUM banks (no more zero-padded ktz halves, no 3.5us
  startup memsets, single k-bias add per cell).
- Normalize fully on-chip: denominator row -> DVE reciprocal (bf16) ->
  K=1 ones-matmul partition-broadcast on the PE -> DVE multiplies. No DRAM
  bounce, no Sync/GpSimd round trips (this removed a ~16us serial tail).
- PV matmuls width-restricted on diagonal blocks (dead prob columns never
  computed; pr memsets gone).
- V projection computed transposed (stationary = x chunk, moving = Wv) so V
  lands token-major directly: no PE transposes, no vtt staging, and bv is
  now added exactly once (post-normalize).
Fully software-pipelined over 512-token chunks t as before; finishes deferred
one pair (drain-keep-1) so the PE never stalls on the DVE reciprocal.
"""
import sys

sys.path.insert(0, "/opt/trn_rl_repo")

import numpy as np
import ml_dtypes

import concourse.bass as bass
import concourse.bacc as bacc
import concourse.tile as tile
import concourse.mybir as mybir
from concourse.bass_utils import run_bass_kernel_spmd

B, S, D, H, HD = 2, 2048, 1024, 16, 64
HPC = 4            # heads per core
HDC = HPC * HD     # 256 hd dims per core
KC = D // 128      # 8 contraction chunks
TQ = S // 512      # 4 q-chunks of 512
SCALE = 1.0 / 8.0  # 1/sqrt(64)

f32 = mybir.dt.float32
bf16 = mybir.dt.bfloat16

_CACHE = {}


def _emit(tc, d, ctx):
    nc = tc.nc
    singles = ctx.enter_context(tc.tile_pool(name="singles", bufs=1))
    xt_pool = ctx.enter_context(tc.tile_pool(name="xt", bufs=3))
    qt_pool = ctx.enter_context(tc.tile_pool(name="qt", bufs=3))
    pr_pool = ctx.enter_context(tc.tile_pool(name="pr", bufs=5))
    norm_pool = ctx.enter_context(tc.tile_pool(name="norm", bufs=3))
    stage_pool = ctx.enter_context(tc.tile_pool(name="stage", bufs=2))
    # PSUM budget (8 banks): psc 2x2 + pv 2 + gen(transients) 2
    gen = ctx.enter_context(tc.tile_pool(name="gen", bufs=2, space="PSUM"))
    psc = ctx.enter_context(tc.tile_pool(name="psc", bufs=2, space="PSUM"))

    xd = {
        nm: d[nm][:].rearrange("(c p) s -> p c s", c=KC) for nm in ("xq", "xk", "xv")
    }

    # --- preamble DMAs: weights/consts on Sync queue, x(0) on GpSimd queue
    w_sb = {}

    def wload(wnm, chunked=False):
        w_sb[wnm] = singles.tile([128, KC, HDC], bf16, tag=wnm, name=wnm)
        src = d[wnm][:].rearrange("p (kc m) -> p kc m", kc=KC)
        if chunked:  # per-chunk DMAs so the first cell matmul can start early
            for c in range(KC):
                nc.sync.dma_start(out=w_sb[wnm][:, c, :], in_=src[:, c, :])
        else:
            nc.sync.dma_start(out=w_sb[wnm], in_=src)

    state = {}

    def prep(t):
        """Allocate x/qt tiles for chunk t and issue x DMAs (GpSimd q)."""
        tsl = slice(t * 512, (t + 1) * 512)
        xts = {}
        for nm in ("xk", "xq", "xv"):
            xts[nm] = xt_pool.tile([128, KC, 512], bf16, tag=nm, name=f"{nm}_{t}")
            if nm == "xv":
                # per-token-block split: v_unit(tb) starts on partial data
                for tb in range(4):
                    nc.gpsimd.dma_start(
                        out=xts[nm][:, :, tb * 128 : (tb + 1) * 128],
                        in_=xd[nm][:, :, t * 512 + tb * 128 : t * 512 + (tb + 1) * 128],
                    )
            elif t == 0:  # chunk-split so the first proj can start early
                for c in range(KC):
                    nc.gpsimd.dma_start(out=xts[nm][:, c, :], in_=xd[nm][:, c, tsl])
            else:
                nc.gpsimd.dma_start(out=xts[nm], in_=xd[nm][:, :, tsl])
        qt = qt_pool.tile([128, 2, 512], bf16, tag="qt", name=f"qt_{t}")
        state[t] = (xts, qt)

    wload("wk", chunked=True)
    prep(0)
    wload("wq", chunked=True)
    bias_sb = singles.tile([128, 6], f32)
    nc.sync.dma_start(out=bias_sb, in_=d["bias"][:])
    trimask = singles.tile([128, 2, 128], bf16)
    nc.sync.dma_start(
        out=trimask, in_=d["trimask"][:].rearrange("p (a b) -> p a b", a=2)
    )
    wload("wv")
    wo_sb = singles.tile([128, 2, D], bf16)
    nc.sync.dma_start(out=wo_sb, in_=d["wo"][:].rearrange("p (c o) -> p c o", c=2))

    # persistent attention state
    ktz_sb = singles.tile([128, 2, S], bf16, tag="ktz")
    # V token-major: [tok 128, key-block, head, 64 hd + 1 ones col (denominator)]
    v_sb = singles.tile([128, S // 128, HPC, 65], bf16, tag="vsb", name="vsb")
    nc.vector.memset(v_sb[:, :, :, 64:65], 1.0)
    ones_sb = singles.tile([65, 64], bf16, tag="ones")
    nc.vector.memset(ones_sb[64:65, :], 1.0)
    attnt_sb = singles.tile([128, 2, S], bf16, tag="attnt")

    def proj_units(t):
        """Yield proj work units for chunk t: 2 k-cells, 2 q-cells, 4 v-chunks."""
        xts, qt = state[t]
        tsl = slice(t * 512, (t + 1) * 512)

        def cell_unit(xnm, wnm, mc, dst_fn):
            def run():
                cell = gen.tile([128, 512], f32, tag="gen", name=f"cell_{wnm}_{t}_{mc}")
                for c in range(KC):
                    nc.tensor.matmul(
                        cell,
                        w_sb[wnm][:, c, mc * 128 : (mc + 1) * 128],
                        xts[xnm][:, c, :],
                        start=(c == 0),
                        stop=(c == KC - 1),
                    )
                dst_fn(mc, cell)

            return run

        def k_dst(mc, cell):
            with nc.allow_low_precision(reason="K in bf16"):
                nc.vector.tensor_scalar_add(
                    out=ktz_sb[:, mc, tsl], in0=cell, scalar1=bias_sb[:, 2 + mc : 3 + mc]
                )

        def q_dst(mc, cell):
            with nc.allow_low_precision(reason="Q in bf16"):
                nc.vector.tensor_scalar_add(
                    out=qt[:, mc, :], in0=cell, scalar1=bias_sb[:, mc : mc + 1]
                )

        def v_unit(tb):
            def run():
                vps = gen.tile([128, 512], f32, tag="gen", name=f"vps_{t}_{tb}")
                for c in range(KC):
                    nc.tensor.matmul(
                        vps[:, 0:HDC],
                        xts["xv"][:, c, tb * 128 : (tb + 1) * 128],
                        w_sb["wv"][:, c, :],
                        start=(c == 0),
                        stop=(c == KC - 1),
                    )
                with nc.allow_low_precision(reason="V in bf16 for PV matmul"):
                    nc.vector.tensor_copy(
                        out=v_sb[:, 4 * t + tb, :, 0:64],
                        in_=vps[:, 0:HDC].rearrange("p (h e) -> p h e", h=HPC),
                    )

            return run

        yield cell_unit("xk", "wk", 0, k_dst)
        yield cell_unit("xq", "wq", 0, q_dst)
        yield v_unit(0)
        yield v_unit(1)
        yield v_unit(2)
        yield cell_unit("xk", "wk", 1, k_dst)
        yield cell_unit("xq", "wq", 1, q_dst)
        yield v_unit(3)

    def outproj_tb(t, tb, final=False):
        def run():
            i = 4 * t + tb
            ot = stage_pool.tile([128, 2, 512], bf16, tag="ot")
            for o in range(2):
                po = gen.tile([128, 512], f32, tag="gen", name=f"po_{i}_{o}")
                for c in range(2):
                    nc.tensor.matmul(
                        po,
                        attnt_sb[:, c, i * 128 : (i + 1) * 128],
                        wo_sb[:, c, o * 512 : (o + 1) * 512],
                        start=(c == 0),
                        stop=(c == 1),
                    )
                nc.vector.tensor_copy(out=ot[:, o, :], in_=po)
            if final:
                # last chunk: spread store halves over idle queues so the
                # closing DMA flush parallelizes instead of draining serially
                nc.sync.dma_start(
                    out=d["out"][i * 128 : (i + 1) * 128, 0:512], in_=ot[:, 0, :]
                )
                nc.scalar.dma_start(
                    out=d["out"][i * 128 : (i + 1) * 128, 512:1024], in_=ot[:, 1, :]
                )
            else:
                nc.gpsimd.dma_start(
                    out=d["out"][i * 128 : (i + 1) * 128, :],
                    in_=ot[:].rearrange("p a b -> p (a b)"),
                )

        return run

    def attention(t, units, p1_units, finishes):
        _, qt = state[t]
        tsl = slice(t * 512, (t + 1) * 512)
        nkb = 4 * t + 4
        for p in range(2):
            # p1 start runs both (t-1) finishes (1-2 pairs old, bounce DMAs
            # long complete) BEFORE outproj(t-1) tb units consume attnt
            while len(finishes) > (2 if p == 0 else 1):
                finishes.pop(0)()
            if p == 1:
                units = p1_units + units
            pvt = [
                gen.tile([128, 512], f32, tag="pv", name=f"pv_{t}_{p}_{h2}")
                for h2 in range(2)
            ]
            pending = None  # software pipeline: PV one kb behind scores
            for kb in range(nkb):
                dg = kb - 4 * t
                lo = 128 * dg if dg > 0 else 0
                scg = psc.tile([128, 2, 512], f32, tag="sc", name=f"sc_{t}_{p}_{kb}")
                # two heads' score matmuls run concurrently: row-tiled K=64
                for h2 in range(2):
                    nc.tensor.matmul(
                        scg[:, h2, lo:512],
                        ktz_sb[h2 * 64 : h2 * 64 + 64, p, kb * 128 : (kb + 1) * 128],
                        qt[h2 * 64 : h2 * 64 + 64, p, lo:512],
                        start=True,
                        stop=True,
                    )
                pr = pr_pool.tile([128, 2, 512], bf16, tag="pr", name=f"pr_{t}_{p}_{kb}")
                nc.scalar.activation(
                    out=pr[:, :, lo:512],
                    in_=scg[:, :, lo:512],
                    func=mybir.ActivationFunctionType.Exp,
                    scale=SCALE,
                )
                if dg >= 0:
                    nc.vector.tensor_tensor(
                        out=pr[:, :, lo : lo + 128],
                        in0=pr[:, :, lo : lo + 128],
                        in1=trimask[:],
                        op=mybir.AluOpType.mult,
                    )
                if pending is not None:
                    pkb, plo, ppr = pending
                    for h2 in range(2):
                        nc.tensor.matmul(
                            pvt[h2][0:65, plo:512],
                            v_sb[:, pkb, 2 * p + h2, :],
                            ppr[:, h2, plo:512],
                            start=(pkb == 0),
                            stop=False,
                        )
                pending = (kb, lo, pr)
                if kb == 1 and mids:
                    mids.pop(0)()
                if units and (p == 0 or kb >= 3 or t == 0):
                    units.pop(0)()
            pkb, plo, ppr = pending
            for h2 in range(2):
                nc.tensor.matmul(
                    pvt[h2][0:65, plo:512],
                    v_sb[:, pkb, 2 * p + h2, :],
                    ppr[:, h2, plo:512],
                    start=(pkb == 0),
                    stop=True,
                )

            # ---- normalize: unnormalized attn rows go to attnt/av1 now;
            # the 1/denom multiplies are DEFERRED two pairs (finishes) so
            # the DVE never head-blocks on the bounce DMA chain. (bv is
            # linear through the normalized attention, so its effect
            # bv @ Wo.T is folded into the host-side bias addition.)
            av1 = norm_pool.tile([64, 512], bf16, tag="av1", name=f"av1_{p}")
            dnr = norm_pool.tile([65, 2, 512], bf16, tag="dnr")
            with nc.allow_low_precision(reason="softmax denom in bf16"):
                for h2 in range(2):
                    nc.vector.tensor_copy(
                        out=dnr[64:65, h2, :], in_=pvt[h2][64:65, :]
                    )
            with nc.allow_low_precision(reason="attn in bf16"):
                nc.vector.tensor_copy(
                    out=attnt_sb[0:64, p, tsl], in_=pvt[0][0:64, :]
                )
                nc.vector.tensor_copy(out=av1[:], in_=pvt[1][0:64, :])
            # unnormalized high half shifted into place EARLY so the
            # partition-shift DMA overlaps the denominator broadcast
            nc.gpsimd.dma_start(out=attnt_sb[64:128, p, tsl], in_=av1[:])
            last = (t, p) == (TQ - 1, 1)
            if last:
                # tail path: broadcast the raw denominators via two
                # col-tiled K=1 ones-matmuls on the (idle) PE; recip+mult
                # run sliced, pipelined with the final outproj.
                bc = gen.tile([128, 512], f32, tag="gen", name="bc_last")
                for h2 in range(2):
                    nc.tensor.matmul(
                        bc[h2 * 64 : h2 * 64 + 64, :],
                        ones_sb[64:65, :],
                        dnr[64:65, h2, :],
                        start=True,
                        stop=True,
                    )
                tail_state.append((p, bc))
                continue
            # pack to [16, 64] so the reciprocal runs 64 elems/lane (DVE
            # reciprocal is ~9 cycles/elem, lane-serial) while the DMA
            # writes 128B/partition (16B/partition scatters take ~8us)
            wide = norm_pool.tile([16, 64], bf16, tag="wide")
            nc.sync.dma_start(out=wide, in_=dnr[64:65, :, :])
            holder = {}

            # three deferred stages, each >= one pair of slack: pack above
            # (inline) -> recip+bounce (next pair kb1) -> multiply (+2 pairs)
            def mid(t=t, p=p, wide=wide, holder=holder):
                wrecp = norm_pool.tile([16, 64], bf16, tag="wrecp")
                with nc.allow_low_precision(reason="softmax denom; bf16 recip"):
                    nc.vector.reciprocal(out=wrecp, in_=wide)
                nc.sync.dma_start(out=d["nscr"][t, p], in_=wrecp)
                bc = norm_pool.tile([128, 512], bf16, tag="bc")
                for h2 in range(2):
                    srcd = d["nscr"][t, p, h2, :]
                    rep = bass.AP(
                        tensor=srcd.tensor,
                        offset=srcd.offset,
                        ap=[[0, 64]] + [list(e) for e in srcd.ap],
                    )
                    nc.sync.dma_start(out=bc[h2 * 64 : h2 * 64 + 64, :], in_=rep)
                holder["bc"] = bc

            mids.append(mid)

            def finish(p=p, holder=holder):
                with nc.allow_low_precision(reason="attn in bf16"):
                    nc.vector.tensor_tensor(
                        out=attnt_sb[:, p, tsl],
                        in0=attnt_sb[:, p, tsl],
                        in1=holder["bc"],
                        op=mybir.AluOpType.mult,
                    )

            finishes.append(finish)
        # flush any proj units not consumed by the kb loops
        while units:
            units.pop(0)()

    # t=0: first 5 proj units up front (k0/q0/v0/v1/v2); the rest interleave
    units0 = list(proj_units(0))
    for u in units0[:5]:
        u()
    finishes = []
    mids = []
    tail_state = []

    # outproj(t-1) tb-blocks interleave into attention(t) p1's kb loop,
    # by which point the normalize chain for (t-1, p1) is long done.
    prep(1)
    for t in range(TQ):
        if t + 2 < TQ:
            prep(t + 2)  # x DMAs prefetched two chunks ahead
        if t + 1 < TQ:
            units = list(proj_units(t + 1))
        else:
            units = []
        if t == 0:
            units = units0[5:] + units
        p1_units = [outproj_tb(t - 1, tb) for tb in range(4)] if t > 0 else []
        attention(t, units, p1_units, finishes)
    while mids:
        mids.pop(0)()
    while finishes:
        finishes.pop(0)()
    # final chunk: last pair's recip+mult sliced per token-block, pipelined
    # with the output projection blocks
    (pl, bcl) = tail_state[0]
    bcr = norm_pool.tile([128, 512], bf16, tag="bcr", name="bcr_last")
    for tb in range(4):
        bsl = slice(tb * 128, (tb + 1) * 128)
        with nc.allow_low_precision(reason="softmax denom; bf16 recip"):
            nc.vector.reciprocal(out=bcr[:, bsl], in_=bcl[:, bsl])
        with nc.allow_low_precision(reason="attn in bf16"):
            nc.vector.tensor_tensor(
                out=attnt_sb[:, pl, (TQ - 1) * 512 + tb * 128 : (TQ - 1) * 512 + (tb + 1) * 128],
                in0=attnt_sb[:, pl, (TQ - 1) * 512 + tb * 128 : (TQ - 1) * 512 + (tb + 1) * 128],
                in1=bcr[:, bsl],
                op=mybir.AluOpType.mult,
            )
        outproj_tb(TQ - 1, tb, final=True)()


def _build_nc():
    nc = bacc.Bacc()
    d = {}
    for nm in ("xq", "xk", "xv"):
        d[nm] = nc.declare_dram_parameter(nm, [D, S], bf16, isOutput=False)
    for nm in ("wq", "wk", "wv"):
        d[nm] = nc.declare_dram_parameter(nm, [128, KC * HDC], bf16, isOutput=False)
    d["wo"] = nc.declare_dram_parameter("wo", [128, 2 * D], bf16, isOutput=False)
    d["bias"] = nc.declare_dram_parameter("bias", [128, 6], f32, isOutput=False)
    d["trimask"] = nc.declare_dram_parameter("trimask", [128, 2 * 128], bf16, isOutput=False)
    d["out"] = nc.declare_dram_parameter("out", [S, D], bf16, isOutput=True)
    d["nscr"] = nc.dram_tensor("nscr", [TQ, 2, 2, 512], bf16)
    from contextlib import ExitStack

    with tile.TileContext(nc) as tc:
        with ExitStack() as ctx:
            _emit(tc, d, ctx)
    nc.compile()
    return nc


def _get_nc():
    if "nc" not in _CACHE:
        _CACHE["nc"] = _build_nc()
    return _CACHE["nc"]


def _xarr(xt):
    return np.ascontiguousarray(xt).astype(ml_dtypes.bfloat16)


def _warr(wt):  # [D, HDC] -> [128, KC*HDC] chunk-contiguous
    return np.ascontiguousarray(
        wt.reshape(KC, 128, HDC).transpose(1, 0, 2).reshape(128, KC * HDC)
    ).astype(ml_dtypes.bfloat16)


def _woarr(wt):  # [HDC, D] -> [128, 2*D]
    return np.ascontiguousarray(
        wt.reshape(2, 128, D).transpose(1, 0, 2).reshape(128, 2 * D)
    ).astype(ml_dtypes.bfloat16)


def _host_consts():
    p = np.arange(128)[:, None]
    j = np.arange(128)[None, :]
    tri = (p <= j).astype(ml_dtypes.bfloat16)
    trimask = np.concatenate([tri, tri], axis=1)  # [128, 2*128], h2-duplicated
    return trimask


def kernel(trace=False, **inputs):
    q = np.asarray(inputs["q"], np.float32)
    k = np.asarray(inputs["k"], np.float32)
    v = np.asarray(inputs["v"], np.float32)
    Wq = np.asarray(inputs["Wq"], np.float32)
    Wk = np.asarray(inputs["Wk"], np.float32)
    Wv = np.asarray(inputs["Wv"], np.float32)
    Wo = np.asarray(inputs["Wo"], np.float32)
    bq = np.asarray(inputs["bq"], np.float32)
    bk = np.asarray(inputs["bk"], np.float32)
    bv = np.asarray(inputs["bv"], np.float32)
    bo = np.asarray(inputs["bo"], np.float32)
    # inputs["mask"] is the causal tril mask, baked into the kernel.

    trimask = _host_consts()
    nc = _get_nc()
    in_maps = []
    for core in range(8):
        b, g = core // 4, core % 4
        sl = slice(g * HDC, (g + 1) * HDC)
        bias = np.zeros((128, 6), np.float32)
        for col, bvec in ((0, bq), (2, bk), (4, bv)):
            seg = bvec[sl].reshape(2, 128)
            bias[:, col] = seg[0]
            bias[:, col + 1] = seg[1]
        in_maps.append(
            {
                "xq": _xarr(q[b].T),
                "xk": _xarr(k[b].T),
                "xv": _xarr(v[b].T),
                "wq": _warr(Wq[sl, :].T),
                "wk": _warr(Wk[sl, :].T),
                "wv": _warr(Wv[sl, :].T),
                "wo": _woarr(Wo[:, sl].T),
                "bias": bias,
                "trimask": trimask,
            }
        )
    res = run_bass_kernel_spmd(nc, in_maps, core_ids=list(range(8)), trace=trace)
    outs = [np.asarray(r["out"], np.float32) for r in res.results]
    final = np.empty((B, S, D), np.float32)
    bias_out = bo + bv @ Wo.T  # bv folded through the output projection
    for b in range(B):
        final[b] = outs[4 * b] + outs[4 * b + 1] + outs[4 * b + 2] + outs[4 * b + 3]
        final[b] += bias_out
    if trace:
        kernel.last_exec_time_ns = res.exec_time_ns
        kernel.last_results = res
    return final
